# revision 25
# baseline (speedup 1.0000x reference)
"""Trainium2 Bass kernel for nn_DMGAGRUcell (GRU cell with graph-conv gates).

Both graph convolutions are weight-folded: z/y projections (node-major,
fp8) let the S/adp fp8-DoubleRow streams accumulate the gate
pre-activations directly in PSUM; sigmoid/tanh apply the inverse
fixed-point scale.  See the 64206ns checkpoint for the classic-gconv1
variant.  LAM_RU is kept small (8192) so z1 stays far from the fp8e4m3
NaN boundary (~464).
"""

import numpy as np
import ml_dtypes

BF16 = ml_dtypes.bfloat16
FP8 = ml_dtypes.float8_e4m3fn

N = 2048
B = 16
D_IN = 2
UNITS = 64
F = 66
B_LOC = 2
N_CORES = 8
KC = 16
KP = 8
NS = 4

S_SCALE = 256.0
A_SCALE = 32768.0
LAM = 32768.0      # c-gate accumulation scale
LAM_RU = 8192.0    # ru-gate accumulation scale (smaller: fp8 z1 headroom)

_CACHE = {}


def _build():
    if "nc" in _CACHE:
        return _CACHE["nc"]

    from contextlib import ExitStack
    import concourse.mybir as mybir
    import concourse.tile as tile
    from concourse import bacc

    f32 = mybir.dt.float32
    bf = mybir.dt.bfloat16
    f8 = mybir.dt.float8e4
    AF = mybir.ActivationFunctionType
    DR = mybir.MatmulPerfMode.DoubleRow

    nc = bacc.Bacc("TRN2", target_bir_lowering=False, debug=False,
                   num_devices=N_CORES)

    adp_d = nc.dram_tensor("adpT", [B_LOC, KP, 128, 2, N], f8, kind="ExternalInput")
    s_d = nc.dram_tensor("sT", [KP, 128, 2, N], f8, kind="ExternalInput")
    hxi_d = nc.dram_tensor("hxi", [B_LOC, F, N], bf, kind="ExternalInput")
    wb_d = nc.dram_tensor("wblob", [F, 576], bf, kind="ExternalInput")
    out_d = nc.dram_tensor("outT", [B_LOC, UNITS, N], bf, kind="ExternalOutput")

    with tile.TileContext(nc) as tc, ExitStack() as ctx:
        cpool = ctx.enter_context(tc.tile_pool(name="cpool", bufs=1))
        spool = ctx.enter_context(tc.tile_pool(name="spool", bufs=1))
        apool = ctx.enter_context(tc.tile_pool(name="apool", bufs=1))
        wk = ctx.enter_context(tc.tile_pool(name="wk", bufs=1))
        pp = ctx.enter_context(tc.tile_pool(name="pp", bufs=4, space="PSUM"))

        def sl(s):
            return slice(s * 512, (s + 1) * 512)

        hxi = [wk.tile([F, N], bf, tag=f"hxi{b}", name=f"hxi{b}")
               for b in range(B_LOC)]
        s2 = [spool.tile([128, 2, N], f8, tag=f"s{kp}", name=f"s{kp}")
              for kp in range(KP)]
        a2 = [[apool.tile([128, 2, N], f8, tag=f"a{b}_{kp}", name=f"a{b}_{kp}")
               for kp in range(KP)] for b in range(B_LOC)]
        wb = cpool.tile([F, 576], bf, tag="wb", name="wb")

        def w_hx(c0, w):
            return wb[0:UNITS, c0:c0 + w]

        def w_inp(c0, w):
            return wb[UNITS:F, c0:c0 + w]

        rhT = [wk.tile([UNITS, N], bf, tag=f"rhT{b}", name=f"rhT{b}") for b in range(B_LOC)]
        ract = [wk.tile([UNITS, N], bf, tag=f"ract{b}", name=f"ract{b}") for b in range(B_LOC)]
        uact = [wk.tile([UNITS, N], bf, tag=f"uact{b}", name=f"uact{b}") for b in range(B_LOC)]
        cT = [wk.tile([UNITS, N], bf, tag=f"cT{b}", name=f"cT{b}") for b in range(B_LOC)]
        outT = [wk.tile([UNITS, N], bf, tag=f"outT{b}", name=f"outT{b}") for b in range(B_LOC)]
        z1nm = [wk.tile([128, KC, 128], f8, tag=f"z1nm{b}", name=f"z1nm{b}")
                for b in range(B_LOC)]
        z2nm = [wk.tile([128, KC, 128], f8, tag=f"z2nm{b}", name=f"z2nm{b}")
                for b in range(B_LOC)]
        y1nm = [wk.tile([128, KC, UNITS], f8, tag=f"y1nm{b}", name=f"y1nm{b}")
                for b in range(B_LOC)]
        y2nm = [wk.tile([128, KC, UNITS], f8, tag=f"y2nm{b}", name=f"y2nm{b}")
                for b in range(B_LOC)]

        nc.sync.dma_start(wb[:], wb_d[:])
        nc.sync.dma_start(hxi[0][:], hxi_d[0])
        for kp in range(KP):
            nc.sync.dma_start(s2[kp][:], s_d[kp])
            if kp == 1:
                # hxi1 isn't needed until the z-b1 projections (~6us in);
                # slotting it here lets s2p0/p1 land ~0.75us earlier and
                # shifts the whole DMA-paced spine forward.
                nc.sync.dma_start(hxi[1][:], hxi_d[1])
        for kp in range(KP):
            nc.sync.dma_start(a2[0][kp][:], adp_d[0, kp])
        for kp in range(KP):
            nc.sync.dma_start(a2[1][kp][:], adp_d[1, kp])

        dum = cpool.tile([1, 2], f32, tag="dum", name="dum")
        nc.scalar.activation(dum[0:1, 0:1], wb[0:1, 0:1], AF.Sigmoid)

        def drain(dst, src, s):
            if s % 2 == 0:
                nc.vector.tensor_copy(dst, src)
            else:
                nc.scalar.activation(dst, src, AF.Copy)

        def proj(b, src_hx, psZ, col, w):
            # node-major projection; ONE accumulation group per PSUM tile
            # (start only on the tile's first matmul) so slot reuse can
            # never leak stale bank contents into the fp8 quantization.
            for k in range(KC):
                ck = slice(128 * k, 128 * (k + 1))
                dst = psZ[k // 4][:, k % 4, :]
                nc.tensor.matmul(dst, hxi[b][UNITS:F, ck], w_inp(col, w),
                                 start=(k % 4 == 0), stop=False)
                nc.tensor.matmul(dst, src_hx[:, ck], w_hx(col, w),
                                 start=False, stop=(k % 4 == 3))

        def znm_copies(z, psZ, s0):
            for q in range(4):
                drain(z[:, 4 * q:4 * q + 4, :], psZ[q][:], q + s0)

        def yproj(b, psY, col, half):
            for k in range(KC):
                ck = slice(128 * k, 128 * (k + 1))
                dst = psY[half * 2 + k // 8][:, k % 8, :]
                nc.tensor.matmul(dst, hxi[b][UNITS:F, ck], w_inp(col, 64),
                                 start=(k % 8 == 0), stop=False)
                nc.tensor.matmul(dst, rhT[b][:, ck], w_hx(col, 64),
                                 start=False, stop=(k % 8 == 7))

        def stream(ps, znm_t, rhs, kps, stop_kp):
            for kp in kps:
                for s in range(NS):
                    nc.tensor.matmul(ps[s][:], znm_t[:, 2 * kp:2 * kp + 2, :],
                                     rhs[kp][:, :, sl(s)],
                                     start=False, stop=(kp == stop_kp),
                                     perf_mode=DR)

        # ---- z projections (front, PE otherwise idle) ----
        psZ1a = [pp.tile([128, 4, 128], f32, tag="pA", name=f"psZ1a_{i}")
                 for i in range(4)]
        proj(0, hxi[0][0:UNITS, :], psZ1a, 128, 128)
        znm_copies(z1nm[0], psZ1a, 0)
        psZ2a = [pp.tile([128, 4, 128], f32, tag="pB", name=f"psZ2a_{i}")
                 for i in range(4)]
        proj(0, hxi[0][0:UNITS, :], psZ2a, 256, 128)
        znm_copies(z2nm[0], psZ2a, 1)
        psZ1b = [pp.tile([128, 4, 128], f32, tag="pA", name=f"psZ1b_{i}")
                 for i in range(4)]
        proj(1, hxi[1][0:UNITS, :], psZ1b, 128, 128)
        znm_copies(z1nm[1], psZ1b, 0)
        psZ2b = [pp.tile([128, 4, 128], f32, tag="pB", name=f"psZ2b_{i}")
                 for i in range(4)]
        proj(1, hxi[1][0:UNITS, :], psZ2b, 256, 128)
        znm_copies(z2nm[1], psZ2b, 1)

        # ---- ru accumulation groups: direct term opens, streams follow ----
        ru0 = [pp.tile([128, 512], f32, tag="pB", name=f"ru0_{s}")
               for s in range(NS)]
        for s in range(NS):
            nc.tensor.matmul(ru0[s][:], w_hx(0, 128), hxi[0][0:UNITS, sl(s)],
                             start=True, stop=False)
            nc.tensor.matmul(ru0[s][:], w_inp(0, 128), hxi[0][UNITS:F, sl(s)],
                             start=False, stop=False)
        ru1 = [pp.tile([128, 512], f32, tag="pA", name=f"ru1_{s}")
               for s in range(NS)]
        for s in range(NS):
            nc.tensor.matmul(ru1[s][:], w_hx(0, 128), hxi[1][0:UNITS, sl(s)],
                             start=True, stop=False)
            nc.tensor.matmul(ru1[s][:], w_inp(0, 128), hxi[1][UNITS:F, sl(s)],
                             start=False, stop=False)
        for kp in range(KP):
            for s in range(NS):
                nc.tensor.matmul(ru0[s][:], z1nm[0][:, 2 * kp:2 * kp + 2, :],
                                 s2[kp][:, :, sl(s)],
                                 start=False, stop=False, perf_mode=DR)
            for s in range(NS):
                nc.tensor.matmul(ru1[s][:], z1nm[1][:, 2 * kp:2 * kp + 2, :],
                                 s2[kp][:, :, sl(s)],
                                 start=False, stop=False, perf_mode=DR)
        stream(ru0, z2nm[0], a2[0], range(KP), KP - 1)

        # ---- b0 gates ----
        for s in range(NS):
            nc.scalar.activation(ract[0][:, sl(s)], ru0[s][0:UNITS, :],
                                 AF.Sigmoid, scale=1.0 / LAM_RU)
            nc.vector.tensor_mul(rhT[0][:, sl(s)], ract[0][:, sl(s)],
                                 hxi[0][0:UNITS, sl(s)])
            nc.scalar.activation(uact[0][:, sl(s)], ru0[s][UNITS:128, :],
                                 AF.Sigmoid, scale=1.0 / LAM_RU)

        def a1_block(kp):
            stream(ru1, z2nm[1], a2[1], [kp], KP - 1)

        a1_block(0)
        psY0 = [pp.tile([128, 8, UNITS], f32, tag="pB", name=f"psY0_{i}")
                for i in range(4)]
        yproj(0, psY0, 384, 0)
        a1_block(1)
        yproj(0, psY0, 448, 1)
        for q in range(4):
            drain(y1nm[0][:, 4 * q:4 * q + 4, :],
                  psY0[q // 2][:, 4 * (q % 2):4 * (q % 2) + 4, :], q)
        a1_block(2)
        for q in range(4):
            drain(y2nm[0][:, 4 * q:4 * q + 4, :],
                  psY0[2 + q // 2][:, 4 * (q % 2):4 * (q % 2) + 4, :], q + 1)

        # ---- b0 gconv2 ----
        psC0 = [pp.tile([UNITS, 512], f32, tag="pB", name=f"psC0_{s}")
                for s in range(NS)]
        for s in range(NS):
            nc.tensor.matmul(psC0[s][:], w_inp(512, 64), hxi[0][UNITS:F, sl(s)],
                             start=True, stop=False)
            nc.tensor.matmul(psC0[s][:], w_hx(512, 64), rhT[0][:, sl(s)],
                             start=False, stop=False)
        for kp in range(KP):
            for s in range(NS):
                nc.tensor.matmul(psC0[s][:], y1nm[0][:, 2 * kp:2 * kp + 2, :],
                                 s2[kp][:, :, sl(s)],
                                 start=False, stop=False, perf_mode=DR)
            if kp == 1:
                a1_block(3)
            if kp == 4:
                a1_block(4)
            if kp == 7:
                a1_block(5)
        for s in range(NS):
            for kp in range(KP):
                nc.tensor.matmul(psC0[s][:], y2nm[0][:, 2 * kp:2 * kp + 2, :],
                                 a2[0][kp][:, :, sl(s)],
                                 start=False, stop=(kp == KP - 1), perf_mode=DR)
            nc.scalar.activation(cT[0][:, sl(s)], psC0[s][:],
                                 AF.Tanh, scale=1.0 / LAM)
            nc.gpsimd.tensor_sub(outT[0][:, sl(s)], hxi[0][0:UNITS, sl(s)],
                                 cT[0][:, sl(s)])
            nc.gpsimd.tensor_mul(outT[0][:, sl(s)], uact[0][:, sl(s)],
                                 outT[0][:, sl(s)])
            nc.gpsimd.tensor_add(outT[0][:, sl(s)], outT[0][:, sl(s)],
                                 cT[0][:, sl(s)])
            if s == 0:
                a1_block(6)
            if s == 1:
                nc.sync.dma_start(out_d[0, :, 0:1024], outT[0][:, 0:1024])
            if s == 2:
                a1_block(7)
            if s == 3:
                nc.sync.dma_start(out_d[0, :, 1024:2048], outT[0][:, 1024:2048])

        # ---- b1 gates ----
        for s in range(NS):
            nc.scalar.activation(ract[1][:, sl(s)], ru1[s][0:UNITS, :],
                                 AF.Sigmoid, scale=1.0 / LAM_RU)
            nc.vector.tensor_mul(rhT[1][:, sl(s)], ract[1][:, sl(s)],
                                 hxi[1][0:UNITS, sl(s)])
            nc.scalar.activation(uact[1][:, sl(s)], ru1[s][UNITS:128, :],
                                 AF.Sigmoid, scale=-1.0 / LAM_RU)
        psY1 = [pp.tile([128, 8, UNITS], f32, tag="pA", name=f"psY1_{i}")
                for i in range(4)]
        yproj(1, psY1, 384, 0)
        yproj(1, psY1, 448, 1)
        for q in range(4):
            drain(y1nm[1][:, 4 * q:4 * q + 4, :],
                  psY1[q // 2][:, 4 * (q % 2):4 * (q % 2) + 4, :], q)
        for q in range(4):
            drain(y2nm[1][:, 4 * q:4 * q + 4, :],
                  psY1[2 + q // 2][:, 4 * (q % 2):4 * (q % 2) + 4, :], q + 1)
        # p = u*hx = hx - u'*hx (u' = 1-u via negated sigmoid)
        for s in range(NS):
            nc.vector.tensor_mul(ract[1][:, sl(s)], uact[1][:, sl(s)],
                                 hxi[1][0:UNITS, sl(s)])
            nc.vector.tensor_sub(ract[1][:, sl(s)], hxi[1][0:UNITS, sl(s)],
                                 ract[1][:, sl(s)])

        # ---- b1 gconv2 ----
        psC1 = [pp.tile([UNITS, 512], f32, tag="pB", name=f"psC1_{s}")
                for s in range(NS)]
        for s in range(NS):
            nc.tensor.matmul(psC1[s][:], w_inp(512, 64), hxi[1][UNITS:F, sl(s)],
                             start=True, stop=False)
            nc.tensor.matmul(psC1[s][:], w_hx(512, 64), rhT[1][:, sl(s)],
                             start=False, stop=False)
        for kp in range(KP):
            for s in range(NS):
                nc.tensor.matmul(psC1[s][:], y1nm[1][:, 2 * kp:2 * kp + 2, :],
                                 s2[kp][:, :, sl(s)],
                                 start=False, stop=False, perf_mode=DR)
        for s in range(NS):
            for kp in range(KP):
                nc.tensor.matmul(psC1[s][:], y2nm[1][:, 2 * kp:2 * kp + 2, :],
                                 a2[1][kp][:, :, sl(s)],
                                 start=False, stop=(kp == KP - 1), perf_mode=DR)
            nc.scalar.activation(cT[1][:, sl(s)], psC1[s][:],
                                 AF.Tanh, scale=1.0 / LAM)
            nc.vector.tensor_mul(outT[1][:, sl(s)], uact[1][:, sl(s)],
                                 cT[1][:, sl(s)])
            nc.vector.tensor_add(outT[1][:, sl(s)], outT[1][:, sl(s)],
                                 ract[1][:, sl(s)])
            if s == 2:
                nc.sync.dma_start(out_d[1, :, 0:1536], outT[1][:, 0:1536])
            if s == 3:
                nc.sync.dma_start(out_d[1, :, 1536:2048], outT[1][:, 1536:2048])

    nc.compile()
    _CACHE["nc"] = nc
    return nc


def _prep_host(inputs, hx, adp, support_rows, support_cols, support_vals,
               W_ru, W_c):
    xcat = np.concatenate(
        [inputs.reshape(B, N, D_IN), hx.reshape(B, N, UNITS)], axis=2)
    xcat = np.ascontiguousarray(xcat, dtype=np.float32)

    S = np.zeros((N, N), np.float32)
    np.add.at(S, (support_rows, support_cols), support_vals)
    s2 = np.ascontiguousarray(
        (S.T * S_SCALE).reshape(KP, 2, 128, N).transpose(0, 2, 1, 3)
    ).astype(FP8)
    adp2 = np.ascontiguousarray(
        (adp.transpose(0, 2, 1) * A_SCALE).reshape(B, KP, 2, 128, N)
        .transpose(0, 1, 3, 2, 4)
    ).astype(FP8)

    xT = xcat.transpose(0, 2, 1)
    hxih = np.concatenate([xT[:, D_IN:F, :], xT[:, 0:D_IN, :]], axis=1)
    hxih = np.ascontiguousarray(hxih).astype(BF16)

    wru = W_ru.reshape(F, 3, 2 * UNITS).astype(np.float32)
    wc = W_c.reshape(F, 3, UNITS).astype(np.float32)
    perm = np.concatenate([np.arange(D_IN, F), np.arange(0, D_IN)])
    wblob = np.zeros((F, 576), np.float32)
    wblob[:, 0:128] = wru[perm, 0, :] * LAM_RU
    wblob[:, 128:256] = wru[perm, 1, :] * (LAM_RU / S_SCALE)
    wblob[:, 256:384] = wru[perm, 2, :] * (LAM_RU / A_SCALE)
    wblob[:, 384:448] = wc[perm, 1, :] * (LAM / S_SCALE)
    wblob[:, 448:512] = wc[perm, 2, :] * (LAM / A_SCALE)
    wblob[:, 512:576] = wc[perm, 0, :] * LAM

    shared = {"sT": s2, "wblob": wblob.astype(BF16)}
    in_maps = []
    for c in range(N_CORES):
        lo, hi = c * B_LOC, (c + 1) * B_LOC
        in_maps.append({
            "adpT": np.ascontiguousarray(adp2[lo:hi]),
            "hxi": np.ascontiguousarray(hxih[lo:hi]),
            **shared,
        })
    return in_maps


def kernel(inputs, hx, adp, support_rows, support_cols, support_vals,
           W_ru, W_c, time_axis=None):
    from concourse.bass_utils import run_bass_kernel_spmd

    inputs = np.asarray(inputs, dtype=np.float32)
    hx = np.asarray(hx, dtype=np.float32)
    adp = np.asarray(adp, dtype=np.float32)
    support_rows = np.asarray(support_rows)
    support_cols = np.asarray(support_cols)
    support_vals = np.asarray(support_vals, dtype=np.float32)
    W_ru = np.asarray(W_ru, dtype=np.float32)
    W_c = np.asarray(W_c, dtype=np.float32)

    nc = _build()
    in_maps = _prep_host(inputs, hx, adp, support_rows, support_cols,
                         support_vals, W_ru, W_c)

    res = run_bass_kernel_spmd(nc, in_maps, core_ids=list(range(N_CORES)),
                               trace=False)
    _CACHE["last_result"] = res

    out = np.empty((B, N * UNITS), np.float32)
    for c in range(N_CORES):
        outT = np.asarray(res.results[c]["outT"], dtype=np.float32)
        for i in range(B_LOC):
            out[c * B_LOC + i] = np.ascontiguousarray(
                outT[i].T).reshape(N * UNITS)
    return out
